# revision 21
# baseline (speedup 1.0000x reference)
"""AFNB (asymmetric fusion non-local block) Trainium2 kernel.

Data-parallel over batch: B=8 images, one per NeuronCore, no collectives.

Per-core algorithm (one image, H*W = N = 4096 pixels):
  pass 1 (low path):  kv = [relu(k_conv); v_conv](low) computed pixel-major in
                      fp8 DoubleRow (2x PE rate), pyramid-pooled (indicator
                      matmul, also fp8 DR) -> kv_pool [110, 512] -> k_pool
                      (PE transpose, stored fp8 at k/4) and v_poolT [110, 256]
                      bf16.  V bias is folded into the final output bias on
                      the host (softmax rows sum to 1, so it is exact).
  pass 2 (per 512-pixel tile): q = relu(q_conv(high)) in fp8 DR at q/4;
                      sim = (k/4).(q/4) (fp8 DR) = k.q/16 BN-folded; softmax
                      over the 110 pooled slots (no max subtraction, |sim|<~5);
                      ctx = v.softmax (bf16); out = A@ctx + W2@high + bias with
                      A = bn_inv*(bn_w1@o_w)/32 and W2 = bn_inv*bn_w2 folded on
                      the host.  W2 matmuls are bf16 (fp8 would fail the error
                      budget); 5 PSUM banks of W2 groups stay open so the PE
                      never idles on the softmax chain; q for tile t+1 is
                      computed mid-tile t.

Scale management for fp8 (e4m3fn; subnormal below 2^-6): kw,vw,kb scaled x32;
qw scaled x256 (undone by the activation scale); k_pool stored as k/4 and q as
q/4 so sim = k.q/16 comes out directly.  Output is stored fp16 (upcast on
host).
"""

import numpy as np
import ml_dtypes

import concourse.bass as bass
import concourse.mybir as mybir
import concourse.tile as tile
from concourse import bacc
from concourse.bass_utils import run_bass_kernel_spmd
from concourse.masks import make_identity

BF = ml_dtypes.bfloat16
F8 = ml_dtypes.float8_e4m3fn
F16 = np.float16
F32 = np.float32
EPS = 1e-5
P = 128
N_CORES = 8
Cl, Ch, Cm, Co = 1024, 2048, 256, 2048
H = W = 64
NPIX = H * W            # 4096
M = 110                 # pooled slots: 1 + 9 + 36 + 64
M2 = 112                # M padded to a multiple of 16 (DoubleRow LDWEIGHTS stride rule)
KO = Cl // P            # 8  low-channel chunks
QO = Ch // P            # 16 high-channel chunks
OC = Co // P            # 16 out-channel chunks
PIX_T = 512             # pixel tile
NT = NPIX // PIX_T      # 8 tiles
PC = PIX_T // P         # 4 pixel chunks (128) per tile
KSC = 32.0              # K/V channel scale for fp8 range
QWS = 256.0             # q-weight fp8 scale
AS8 = 256.0             # A fp8 scale (A8@ctx32 = 8192*A@ctx)
WSC = 8192.0            # W2 scale matching the A8/ctx32 psum scale

_cached = None
_last_results = None


def _pool_matrix(in_size, out_size):
    Pm = np.zeros((out_size, in_size), np.float64)
    for i in range(out_size):
        s = (i * in_size) // out_size
        e = -((-(i + 1) * in_size) // out_size)
        Pm[i, s:e] = 1.0 / (e - s)
    return Pm


def _build_pool_indicator():
    rows, areas = [], []
    for s in (1, 3, 6, 8):
        Ph = _pool_matrix(H, s) != 0
        Pw = _pool_matrix(W, s) != 0
        for i in range(s):
            for j in range(s):
                ind = np.outer(Ph[i], Pw[j]).reshape(-1)
                rows.append(ind.astype(np.float32))
                areas.append(ind.sum())
    return np.stack(rows), 1.0 / np.asarray(areas, np.float64)


def _chunk_T(w, chunks):
    """[rows, cols] -> SBUF layout [128, chunks, rows] with [p,o,m]=w[m,o*128+p]."""
    rows, cols = w.shape
    assert cols == chunks * P
    return np.ascontiguousarray(w.T.reshape(chunks, P, rows).transpose(1, 0, 2))


def _prep_weights(inp):
    f64 = lambda k: np.asarray(inp[k], np.float64)
    inv_q = f64("q_g") / np.sqrt(f64("q_v") + EPS)
    qw = (inv_q[:, None] * f64("q_w")) / 16.0        # fold BN + 1/sqrt(256)
    qb = (f64("q_b") - f64("q_m") * inv_q) / 16.0
    inv_k = f64("k_g") / np.sqrt(f64("k_v") + EPS)
    kw = inv_k[:, None] * f64("k_w")
    kb = f64("k_b") - f64("k_m") * inv_k
    bn_w1 = f64("bn_w")[:, :Ch]
    bn_w2 = f64("bn_w")[:, Ch:]
    inv_bn = f64("bn_g") / np.sqrt(f64("bn_v") + EPS)
    A = inv_bn[:, None] * (bn_w1 @ f64("o_w"))       # [2048, 256]
    W2 = inv_bn[:, None] * bn_w2                     # [2048, 2048]
    # v_b folded here: ctx = softmax@(v+v_b) = softmax@v + v_b exactly
    bias_out = (inv_bn * (bn_w1 @ (f64("o_w") @ f64("v_b") + f64("o_b")))
                + f64("bn_b") - f64("bn_m") * inv_bn)

    kv_w = np.concatenate([kw, f64("v_w")], 0) * KSC  # [512, 1024] x32
    ind, area_recip = _build_pool_indicator()         # [110, 4096], [110]
    ind = np.concatenate([ind, np.zeros((M2 - M, NPIX), np.float32)], 0)

    # W2 reordered oc-major for contiguous per-oc-chunk streaming DMAs:
    # [128, OC, QO, 128] with [p, oc, o, c] = W2[oc*128+c, o*128+p]
    w2T = (W2.T.reshape(QO, P, OC, P).transpose(1, 2, 0, 3))

    return {
        "kvw8": _chunk_T(kv_w, KO).astype(F8),                        # [128, 8, 512]
        "kb": np.ascontiguousarray(np.stack([(kb * KSC), np.zeros_like(kb)])[None]).astype(F8),  # [1, 2, 256]
        "qw8": _chunk_T(qw * QWS, QO).astype(F8),                     # [128, 16, 256]
        "qb": np.ascontiguousarray((qb / 4.0).reshape(2, P).T).astype(F32),  # [128, 2]
        "AT": _chunk_T(A * AS8, 2).astype(F8),                        # [128, 2, 2048]
        "W2T": np.ascontiguousarray(w2T * WSC).astype(BF),            # [128, 16, 16, 128]
        "bout": np.ascontiguousarray(bias_out.reshape(OC, P).T).astype(F32),  # [128, 16]
        "ind8": _chunk_T(ind, NPIX // P).astype(F8),                  # [128, 32, 112]
        "area": np.ascontiguousarray(area_recip[:, None]).astype(F32),  # [110, 1]
    }


def build_bass():
    bf = mybir.dt.bfloat16
    f8 = mybir.dt.float8e4
    f16 = mybir.dt.float16
    f32 = mybir.dt.float32
    DR = mybir.MatmulPerfMode.DoubleRow
    ACT = mybir.ActivationFunctionType
    nc = bacc.Bacc()
    low_e = nc.declare_dram_parameter("low", [P, NT, KO, PIX_T], f8, isOutput=False)
    high_e = nc.declare_dram_parameter("high", [P, NT, QO, PIX_T], bf, isOutput=False)
    kvw_e = nc.declare_dram_parameter("kvw8", [P, KO, 512], f8, isOutput=False)
    kb_e = nc.declare_dram_parameter("kb", [1, 2, Cm], f8, isOutput=False)
    qw_e = nc.declare_dram_parameter("qw8", [P, QO, Cm], f8, isOutput=False)
    qb_e = nc.declare_dram_parameter("qb", [P, 2], f32, isOutput=False)
    at_e = nc.declare_dram_parameter("AT", [P, 2, Co], f8, isOutput=False)
    w2_e = nc.declare_dram_parameter("W2T", [P, OC, QO, P], bf, isOutput=False)
    bo_e = nc.declare_dram_parameter("bout", [P, OC], f32, isOutput=False)
    ind_e = nc.declare_dram_parameter("ind8", [P, NPIX // P, M2], f8, isOutput=False)
    ar_e = nc.declare_dram_parameter("area", [M, 1], f32, isOutput=False)
    out_e = nc.declare_dram_parameter("out", [Co, NPIX], f16, isOutput=True)

    out_r = out_e[:].rearrange("(o p) n -> o p n", p=P)    # [16, 128, 4096]

    with tile.TileContext(nc) as tc:
        with (
            tc.tile_pool(name="consts", bufs=1) as consts,
            tc.tile_pool(name="lobf", bufs=3) as lobf_p,
            tc.tile_pool(name="kvt", bufs=2) as kvt_p,
            tc.tile_pool(name="hibf", bufs=2) as hibf_p,
            tc.tile_pool(name="hi8b", bufs=2) as hi8_p,
            tc.tile_pool(name="qsb", bufs=2) as q_p,
            tc.tile_pool(name="esb", bufs=1) as e_p,
            tc.tile_pool(name="ensb", bufs=1) as en_p,
            tc.tile_pool(name="rsb", bufs=1) as r_p,
            tc.tile_pool(name="ctxsb", bufs=2) as ctx_p,
            tc.tile_pool(name="osb", bufs=3) as o_p,
            tc.tile_pool(name="psbig", bufs=2, space="PSUM") as psbig_p,
            tc.tile_pool(name="pso", bufs=5, space="PSUM") as pso_p,
            tc.tile_pool(name="psmall", bufs=1, space="PSUM") as psmall_p,
        ):
            # pass-1 consts first: their DMAs go ahead of the fat pass-2 weights
            kb_sb = consts.tile([1, 2, Cm], f8)
            nc.sync.dma_start(kb_sb, kb_e[:])
            kvw_sb = consts.tile([P, KO, 512], f8)
            for o2 in range(KO // 2):   # pair-granular so conv starts early
                nc.sync.dma_start(kvw_sb[:, 2 * o2:2 * o2 + 2, :],
                                  kvw_e[:][:, 2 * o2:2 * o2 + 2, :])
            ind_sb = consts.tile([P, NPIX // P, M2], f8)
            nc.gpsimd.dma_start(ind_sb[:, 0:16, :], ind_e[:][:, 0:16, :])
            nc.gpsimd.dma_start(ind_sb[:, 16:32, :], ind_e[:][:, 16:32, :])
            ar_sb = consts.tile([M, 1], f32)
            nc.gpsimd.dma_start(ar_sb, ar_e[:])
            qw_sb = consts.tile([P, QO, Cm], f8)
            nc.gpsimd.dma_start(qw_sb, qw_e[:])
            qb_sb = consts.tile([P, 2], f32)
            nc.gpsimd.dma_start(qb_sb, qb_e[:])

            ones8 = consts.tile([1, 2, P], f8)    # K-bias DR seed lhsT
            nc.vector.memset(ones8[:, 0, :], 1.0)
            nc.vector.memset(ones8[:, 1, :], 0.0)
            ones1m = consts.tile([1, M], bf)      # psr broadcast lhsT
            nc.vector.memset(ones1m, 1.0)
            ones_m = consts.tile([M, 1], bf)      # denominator lhsT
            nc.vector.memset(ones_m, 1.0)
            ident = consts.tile([P, P], f32)
            make_identity(nc, ident)

            kvpool_f32 = consts.tile([M, 512], f32)
            v_poolT = consts.tile([M, Cm], bf)
            k_pool8 = consts.tile([P, 2, M2], f8)
            nc.vector.memset(k_pool8, 0.0)

            # ---------------- pass 1: low -> pooled K/V (fp8 DR) ----------------
            pool_acc = pso_p.tile([M2, 512], f32, name="pool_acc", tag="o")
            kvt8 = None
            kvt_gate = None
            for dt_ in range(NT):
                lo8 = lobf_p.tile([P, KO, PIX_T], f8)
                if dt_ == 0:   # tile 0 pair-split on the scalar queue
                    for o2 in range(KO // 2):
                        nc.scalar.dma_start(lo8[:, 2 * o2:2 * o2 + 2, :],
                                            low_e[:][:, 0, 2 * o2:2 * o2 + 2, :])
                elif dt_ % 2 == 1:   # alternate queues: 2x streaming bandwidth
                    nc.scalar.dma_start(lo8, low_e[:][:, dt_])
                else:
                    nc.sync.dma_start(lo8, low_e[:][:, dt_])
                for tt in range(PC):
                    t = dt_ * PC + tt
                    ps = psbig_p.tile([P, 512], f32, tag="big")
                    for o2 in range(KO // 2):
                        nc.tensor.matmul(
                            ps, lo8[:, 2 * o2:2 * o2 + 2, tt * P:(tt + 1) * P],
                            kvw_sb[:, 2 * o2:2 * o2 + 2, :],
                            start=(o2 == 0), stop=(o2 == KO // 2 - 1),
                            perf_mode=DR, skip_group_check=True)
                        if o2 == 0:   # K-bias into the zeroed psum (cols 0:256)
                            nc.tensor.matmul(ps[:, 0:Cm], ones8[:, 0:2, :],
                                             kb_sb[:, 0:2, :],
                                             start=False, stop=False, perf_mode=DR,
                                             skip_group_check=True)
                    half = t % 2
                    if half == 0:
                        kvt8 = kvt_p.tile([P, 2, 512], f8)
                    nc.scalar.activation(kvt8[:, half, 0:Cm], ps[:, 0:Cm], ACT.Relu)
                    nc.vector.tensor_copy(kvt8[:, half, Cm:512], ps[:, Cm:512])
                    if half == 1:
                        nc.tensor.matmul(pool_acc, ind_sb[:, t - 1:t + 1, :], kvt8,
                                         start=(t == 1), stop=(t == NPIX // P - 1),
                                         perf_mode=DR, skip_group_check=True)
                        if t == 11:
                            kvt_gate = kvt8

            # prefetch tile-0 high (bf16 on sync q, fp8 on vector q)
            hi_tiles, hi8_tiles = {}, {}

            def stage_hi(t):
                hi_bf = hibf_p.tile([P, QO, PIX_T], bf)
                if t == 0:   # halves so tile-0 W2 opens start sooner
                    nc.sync.dma_start(hi_bf[:, 0:QO // 2, :],
                                      high_e[:][:, 0, 0:QO // 2, :])
                    nc.sync.dma_start(hi_bf[:, QO // 2:QO, :],
                                      high_e[:][:, 0, QO // 2:QO, :])
                else:
                    nc.sync.dma_start(hi_bf, high_e[:][:, t])
                hi_tiles[t] = hi_bf

            def cast_hi8(t):
                # on-chip bf16 -> fp8 cast for the q conv: saves 8 MB of DMA
                hi_bf = hi_tiles[t]
                hi8 = hi8_p.tile([P, QO, PIX_T], f8)
                nc.vector.tensor_copy(hi8[:, 0:QO // 2, :], hi_bf[:, 0:QO // 2, :])
                nc.vector.tensor_copy(hi8[:, QO // 2:QO, :], hi_bf[:, QO // 2:QO, :])
                hi8_tiles[t] = hi8

            stage_hi(0)
            cast_hi8(0)

            # fat pass-2 weights: qw8/at first (needed at tile-0 start), then
            # W2 streamed per oc chunk so tile-0 opens consume them as they land
            at_sb = consts.tile([P, 2, Co], f8)
            nc.gpsimd.dma_start(at_sb, at_e[:])
            w2gate = consts.tile([1, 1], f8)
            nc.gpsimd.tensor_copy(w2gate, kvt_gate[0:1, 0, 0:1])
            w2_sb = consts.tile([P, OC, QO, P], bf)
            for oc in range(2):
                nc.gpsimd.dma_start(w2_sb[:, oc], w2_e[:][:, oc])
            bo_sb = consts.tile([P, OC], f32)
            nc.gpsimd.dma_start(bo_sb, bo_e[:])

            def emit_q(t):
                """fp8 DR q conv for tile t -> q/4 in fp8 [P, 2, PIX_T]."""
                hi8 = hi8_tiles[t]
                q8 = q_p.tile([P, 2, PIX_T], f8)
                for j in range(2):
                    psq = psbig_p.tile([P, PIX_T], f32, tag="big")
                    for o2 in range(QO // 2):
                        nc.tensor.matmul(
                            psq, qw_sb[:, 2 * o2:2 * o2 + 2, j * P:(j + 1) * P],
                            hi8[:, 2 * o2:2 * o2 + 2, :],
                            start=(o2 == 0), stop=(o2 == QO // 2 - 1),
                            perf_mode=DR)
                    nc.scalar.activation(q8[:, j, :], psq, ACT.Relu,
                                         bias=qb_sb[:, j:j + 1], scale=1.0 / (4.0 * QWS))
                return q8

            q_next = emit_q(0)
            # rest of W2 gated on tile-0 q conv: keeps the 15-30us DMA window
            # free for hi8[0]/hi_bf[0], while W2 still lands ahead of its opens
            w2gate2 = consts.tile([1, 1], f8)
            nc.gpsimd.tensor_copy(w2gate2, q_next[0:1, 0, 0:1])
            for oc in range(2, OC):
                nc.gpsimd.dma_start(w2_sb[:, oc], w2_e[:][:, oc])

            # epilogue: scale by 1/area, split V (bf16) / K (fp8 at k/4 via
            # transpose + scaled copy); PE transposes overlap tile-0 q conv
            nc.vector.tensor_scalar_mul(kvpool_f32, pool_acc[0:M, :], ar_sb)
            nc.scalar.activation(v_poolT, kvpool_f32[:, Cm:512], ACT.Copy)
            for j in range(2):
                pst = psbig_p.tile([P, M], f32, tag="big")
                nc.tensor.transpose(pst, kvpool_f32[:, j * P:(j + 1) * P],
                                    ident[:M, :M])
                nc.scalar.activation(k_pool8[:, j, 0:M], pst, ACT.Identity,
                                     scale=1.0 / (4.0 * KSC))

            # ---------------- pass 2: per pixel tile ----------------
            for t in range(NT):
                sl = slice(t * PIX_T, (t + 1) * PIX_T)
                hi_bf = hi_tiles.pop(t)
                hi8_tiles.pop(t)
                if t + 1 < NT:
                    stage_hi(t + 1)
                stage_t = t
                q8 = q_next

                psim = psmall_p.tile([M2, PIX_T], f32, tag="s")
                nc.tensor.matmul(psim, k_pool8[:, 0:2, :], q8[:, 0:2, :],
                                 start=True, stop=True, perf_mode=DR,
                                 skip_group_check=True)
                e_sb = e_p.tile([M, PIX_T], bf)
                nc.scalar.activation(e_sb, psim[0:M, :], ACT.Exp)

                pso_tiles = {}

                def open_half(oc, hf, hi_bf=hi_bf):
                    if hf == 0:
                        pso_tiles[oc] = pso_p.tile([P, PIX_T], f32, name="pso", tag="o")
                    pso = pso_tiles[oc]
                    for o in range(hf * (QO // 2), (hf + 1) * (QO // 2)):
                        nc.tensor.matmul(pso, w2_sb[:, oc, o, :], hi_bf[:, o, :],
                                         start=(o == 0), stop=False,
                                         skip_group_check=True)

                def close_group(oc, sl=sl, t=t):
                    pso = pso_tiles.pop(oc)
                    ctx_sb = ctx_cur[0]
                    nc.tensor.matmul(pso, at_sb[:, 0:2, oc * P:(oc + 1) * P],
                                     ctx_sb[:, 0:2, :],
                                     start=False, stop=True, perf_mode=DR,
                                     skip_group_check=True)
                    o_sb = o_p.tile([P, PIX_T], f16)
                    if t == NT - 1 and oc >= OC - 4:
                        # final drain: both engines + both DMA queues in halves
                        h = PIX_T // 2
                        base = t * PIX_T
                        nc.scalar.activation(o_sb[:, 0:h], pso[:, 0:h],
                                             ACT.Identity,
                                             bias=bo_sb[:, oc:oc + 1],
                                             scale=1.0 / WSC)
                        nc.vector.tensor_scalar(o_sb[:, h:], pso[:, h:],
                                                1.0 / WSC, bo_sb[:, oc:oc + 1],
                                                mybir.AluOpType.mult,
                                                mybir.AluOpType.add)
                        nc.sync.dma_start(out_r[oc][:, base:base + h],
                                          o_sb[:, 0:h])
                        nc.scalar.dma_start(out_r[oc][:, base + h:base + PIX_T],
                                            o_sb[:, h:])
                        return
                    if oc % 2 == 1:
                        nc.scalar.activation(o_sb, pso, ACT.Identity,
                                             bias=bo_sb[:, oc:oc + 1], scale=1.0 / WSC)
                    else:
                        nc.vector.tensor_scalar(o_sb, pso, 1.0 / WSC,
                                                bo_sb[:, oc:oc + 1],
                                                mybir.AluOpType.mult,
                                                mybir.AluOpType.add)
                    nc.sync.dma_start(out_r[oc][:, sl], o_sb)

                open_half(0, 0)
                psd = psmall_p.tile([1, PIX_T], f32, tag="s")
                nc.tensor.matmul(psd, ones_m, e_sb, start=True, stop=True,
                                 skip_group_check=True)
                open_half(0, 1)
                open_half(1, 0)
                r_sb = r_p.tile([1, PIX_T], f32)
                nc.vector.reciprocal_approx_fast(out=r_sb, in_=psd)
                r_bf = r_p.tile([1, PIX_T], bf, name="r_bf")
                nc.vector.tensor_copy(r_bf, r_sb)
                psr = psmall_p.tile([M, PIX_T], f32, tag="s")
                nc.tensor.matmul(psr, ones1m, r_bf, start=True, stop=True,
                                 skip_group_check=True)
                open_half(1, 1)
                open_half(2, 0)
                en_sb = en_p.tile([M, PIX_T], bf)
                nc.vector.tensor_mul(en_sb, e_sb, psr)
                ctx_sb = ctx_p.tile([P, 2, PIX_T], f8)
                ctx_cur = [ctx_sb]
                psc0 = psbig_p.tile([P, PIX_T], f32, tag="big")
                nc.tensor.matmul(psc0, v_poolT[:, 0:P], en_sb,
                                 start=True, stop=True, skip_group_check=True)
                psc1 = psbig_p.tile([P, PIX_T], f32, tag="big")
                nc.tensor.matmul(psc1, v_poolT[:, P:2 * P], en_sb,
                                 start=True, stop=True, skip_group_check=True)
                open_half(2, 1)
                open_half(3, 0)
                nc.scalar.activation(ctx_sb[:, 0, :], psc0, ACT.Copy)
                nc.vector.tensor_copy(ctx_sb[:, 1, :], psc1)
                if stage_t + 1 < NT:
                    cast_hi8(stage_t + 1)
                open_half(3, 1)
                open_half(4, 0)
                open_half(4, 1)
                close_group(0)
                close_group(1)
                # q conv for the next tile: independent PE work that also
                # covers close(0)/(1)'s psum drain before bank reuse
                if t + 1 < NT:
                    q_next = emit_q(t + 1)
                open_half(5, 0)
                open_half(5, 1)
                for i in range(2, OC - 4):
                    close_group(i)
                    if i + 4 < OC:
                        open_half(i + 4, 0)
                        open_half(i + 4, 1)
                for oc in range(OC - 4, OC):
                    close_group(oc)
    nc.finalize()
    return nc


def kernel(**inputs):
    global _cached, _last_results
    if _cached is None:
        _cached = build_bass()
    nc = _cached
    wts = _prep_weights(inputs)
    # pack [C, H*W] -> [p, tile, o, pix] so each per-tile DMA is contiguous
    low = np.ascontiguousarray(
        np.asarray(inputs["low_feats"], F32).reshape(N_CORES, KO, P, NT, PIX_T)
        .transpose(0, 2, 3, 1, 4).astype(F8))
    high_f = (np.asarray(inputs["high_feats"], F32)
              .reshape(N_CORES, QO, P, NT, PIX_T).transpose(0, 2, 3, 1, 4))
    high = np.ascontiguousarray(high_f.astype(BF))
    in_maps = [dict(wts, low=low[i], high=high[i])
               for i in range(N_CORES)]
    res = run_bass_kernel_spmd(nc, in_maps, core_ids=list(range(N_CORES)))
    _last_results = res
    out = np.stack([res.results[i]["out"] for i in range(N_CORES)])
    return out.reshape(N_CORES, Co, H, W).astype(F32)


if __name__ == "__main__":
    rng = np.random.default_rng(0)
    dummy = {
        "low_feats": rng.standard_normal((8, Cl, H, W), dtype=np.float32),
        "high_feats": rng.standard_normal((8, Ch, H, W), dtype=np.float32),
    }
    for k, shape in [("q_w", (Cm, Ch)), ("k_w", (Cm, Cl)), ("v_w", (Cm, Cl)),
                     ("o_w", (Co, Cm)), ("bn_w", (Co, Co + Ch))]:
        dummy[k] = rng.standard_normal(shape, dtype=np.float32) * 0.02
    for k in ["q_g", "q_v", "k_g", "k_v"]:
        dummy[k] = rng.uniform(0.5, 1.5, Cm).astype(np.float32)
    for k in ["q_b", "q_m", "k_b", "k_m", "v_b"]:
        dummy[k] = rng.standard_normal(Cm).astype(np.float32) * 0.1
    for k in ["bn_g", "bn_v"]:
        dummy[k] = rng.uniform(0.5, 1.5, Co).astype(np.float32)
    for k in ["bn_b", "bn_m", "o_b"]:
        dummy[k] = rng.standard_normal(Co).astype(np.float32) * 0.1
    out = kernel(**dummy)
    print("out", out.shape, out.dtype)


# revision 22
# speedup vs baseline: 1.1999x; 1.1999x over previous
"""AFNB (asymmetric fusion non-local block) Trainium2 kernel.

Data-parallel over batch: B=8 images, one per NeuronCore, no collectives.

Per-core algorithm (one image, H*W = N = 4096 pixels):
  pass 1 (low path):  kv = [relu(k_conv); v_conv](low) computed pixel-major in
                      fp8 DoubleRow (2x PE rate), pyramid-pooled (indicator
                      matmul, also fp8 DR) -> kv_pool [110, 512] -> k_pool
                      (PE transpose, stored fp8 at k/4) and v_poolT [110, 256]
                      bf16.  V bias is folded into the final output bias on
                      the host (softmax rows sum to 1, so it is exact).
  pass 2 (per 512-pixel tile): q = relu(q_conv(high)) in fp8 DR at q/4;
                      sim = (k/4).(q/4) (fp8 DR) = k.q/16 BN-folded; softmax
                      over the 110 pooled slots (no max subtraction, |sim|<~5);
                      ctx = v.softmax (bf16); out = A@ctx + W2@high + bias with
                      A = bn_inv*(bn_w1@o_w)/32 and W2 = bn_inv*bn_w2 folded on
                      the host.  W2 matmuls are bf16 (fp8 would fail the error
                      budget); 5 PSUM banks of W2 groups stay open so the PE
                      never idles on the softmax chain; q for tile t+1 is
                      computed mid-tile t.

Scale management for fp8 (e4m3fn; subnormal below 2^-6): kw,vw,kb scaled x32;
qw scaled x256 (undone by the activation scale); k_pool stored as k/4 and q as
q/4 so sim = k.q/16 comes out directly.  Output is stored fp16 (upcast on
host).
"""

import numpy as np
import ml_dtypes

import concourse.bass as bass
import concourse.mybir as mybir
import concourse.tile as tile
from concourse import bacc
from concourse.bass_utils import run_bass_kernel_spmd
from concourse.masks import make_identity

BF = ml_dtypes.bfloat16
F8 = ml_dtypes.float8_e4m3fn
F16 = np.float16
F32 = np.float32
EPS = 1e-5
P = 128
N_CORES = 8
Cl, Ch, Cm, Co = 1024, 2048, 256, 2048
H = W = 64
NPIX = H * W            # 4096
M = 110                 # pooled slots: 1 + 9 + 36 + 64
M2 = 112                # M padded to a multiple of 16 (DoubleRow LDWEIGHTS stride rule)
KO = Cl // P            # 8  low-channel chunks
QO = Ch // P            # 16 high-channel chunks
OC = Co // P            # 16 out-channel chunks
PIX_T = 512             # pixel tile
NT = NPIX // PIX_T      # 8 tiles
PC = PIX_T // P         # 4 pixel chunks (128) per tile
KSC = 32.0              # K/V channel scale for fp8 range
QWS = 256.0             # q-weight fp8 scale
AS8 = 256.0             # A fp8 scale (A8@ctx32 = 8192*A@ctx)
WSC = 8192.0            # W2 scale matching the A8/ctx32 psum scale

_cached = None
_last_results = None


def _pool_matrix(in_size, out_size):
    Pm = np.zeros((out_size, in_size), np.float64)
    for i in range(out_size):
        s = (i * in_size) // out_size
        e = -((-(i + 1) * in_size) // out_size)
        Pm[i, s:e] = 1.0 / (e - s)
    return Pm


def _build_pool_indicator():
    rows, areas = [], []
    for s in (1, 3, 6, 8):
        Ph = _pool_matrix(H, s) != 0
        Pw = _pool_matrix(W, s) != 0
        for i in range(s):
            for j in range(s):
                ind = np.outer(Ph[i], Pw[j]).reshape(-1)
                rows.append(ind.astype(np.float32))
                areas.append(ind.sum())
    return np.stack(rows), 1.0 / np.asarray(areas, np.float64)


def _chunk_T(w, chunks):
    """[rows, cols] -> SBUF layout [128, chunks, rows] with [p,o,m]=w[m,o*128+p]."""
    rows, cols = w.shape
    assert cols == chunks * P
    return np.ascontiguousarray(w.T.reshape(chunks, P, rows).transpose(1, 0, 2))


def _prep_weights(inp):
    f64 = lambda k: np.asarray(inp[k], np.float64)
    inv_q = f64("q_g") / np.sqrt(f64("q_v") + EPS)
    qw = (inv_q[:, None] * f64("q_w")) / 16.0        # fold BN + 1/sqrt(256)
    qb = (f64("q_b") - f64("q_m") * inv_q) / 16.0
    inv_k = f64("k_g") / np.sqrt(f64("k_v") + EPS)
    kw = inv_k[:, None] * f64("k_w")
    kb = f64("k_b") - f64("k_m") * inv_k
    bn_w1 = f64("bn_w")[:, :Ch]
    bn_w2 = f64("bn_w")[:, Ch:]
    inv_bn = f64("bn_g") / np.sqrt(f64("bn_v") + EPS)
    A = inv_bn[:, None] * (bn_w1 @ f64("o_w"))       # [2048, 256]
    W2 = inv_bn[:, None] * bn_w2                     # [2048, 2048]
    # v_b folded here: ctx = softmax@(v+v_b) = softmax@v + v_b exactly
    bias_out = (inv_bn * (bn_w1 @ (f64("o_w") @ f64("v_b") + f64("o_b")))
                + f64("bn_b") - f64("bn_m") * inv_bn)

    kv_w = np.concatenate([kw, f64("v_w")], 0) * KSC  # [512, 1024] x32
    ind, area_recip = _build_pool_indicator()         # [110, 4096], [110]
    ind = np.concatenate([ind, np.zeros((M2 - M, NPIX), np.float32)], 0)

    # W2 reordered oc-major for contiguous per-oc-chunk streaming DMAs:
    # [128, OC, QO, 128] with [p, oc, o, c] = W2[oc*128+c, o*128+p]
    w2T = (W2.T.reshape(QO, P, OC, P).transpose(1, 2, 0, 3))

    return {
        "kvw8": _chunk_T(kv_w, KO).astype(F8),                        # [128, 8, 512]
        "kb": np.ascontiguousarray(np.stack([(kb * KSC), np.zeros_like(kb)])[None]).astype(F8),  # [1, 2, 256]
        "qw8": _chunk_T(qw * QWS, QO).astype(F8),                     # [128, 16, 256]
        "qb": np.ascontiguousarray((qb / 4.0).reshape(2, P).T).astype(F32),  # [128, 2]
        "AT": _chunk_T(A * AS8, 2).astype(F8),                        # [128, 2, 2048]
        "W2T": np.ascontiguousarray(w2T * WSC).astype(BF),            # [128, 16, 16, 128]
        "bout": np.ascontiguousarray(bias_out.reshape(OC, P).T).astype(F32),  # [128, 16]
        "ind8": _chunk_T(ind, NPIX // P).astype(F8),                  # [128, 32, 112]
        "area": np.ascontiguousarray(area_recip[:, None]).astype(F32),  # [110, 1]
    }


def build_bass():
    bf = mybir.dt.bfloat16
    f8 = mybir.dt.float8e4
    f16 = mybir.dt.float16
    f32 = mybir.dt.float32
    DR = mybir.MatmulPerfMode.DoubleRow
    ACT = mybir.ActivationFunctionType
    nc = bacc.Bacc()
    low_e = nc.declare_dram_parameter("low", [P, NT, KO, PIX_T], f8, isOutput=False)
    high_e = nc.declare_dram_parameter("high", [P, NT, QO, PIX_T], bf, isOutput=False)
    kvw_e = nc.declare_dram_parameter("kvw8", [P, KO, 512], f8, isOutput=False)
    kb_e = nc.declare_dram_parameter("kb", [1, 2, Cm], f8, isOutput=False)
    qw_e = nc.declare_dram_parameter("qw8", [P, QO, Cm], f8, isOutput=False)
    qb_e = nc.declare_dram_parameter("qb", [P, 2], f32, isOutput=False)
    at_e = nc.declare_dram_parameter("AT", [P, 2, Co], f8, isOutput=False)
    w2_e = nc.declare_dram_parameter("W2T", [P, OC, QO, P], bf, isOutput=False)
    bo_e = nc.declare_dram_parameter("bout", [P, OC], f32, isOutput=False)
    ind_e = nc.declare_dram_parameter("ind8", [P, NPIX // P, M2], f8, isOutput=False)
    ar_e = nc.declare_dram_parameter("area", [M, 1], f32, isOutput=False)
    out_e = nc.declare_dram_parameter("out", [Co, NPIX], f16, isOutput=True)

    out_r = out_e[:].rearrange("(o p) n -> o p n", p=P)    # [16, 128, 4096]

    with tile.TileContext(nc) as tc:
        with (
            tc.tile_pool(name="consts", bufs=1) as consts,
            tc.tile_pool(name="lobf", bufs=3) as lobf_p,
            tc.tile_pool(name="kvt", bufs=2) as kvt_p,
            tc.tile_pool(name="hibf", bufs=2) as hibf_p,
            tc.tile_pool(name="hi8b", bufs=2) as hi8_p,
            tc.tile_pool(name="qsb", bufs=2) as q_p,
            tc.tile_pool(name="esb", bufs=1) as e_p,
            tc.tile_pool(name="ensb", bufs=1) as en_p,
            tc.tile_pool(name="rsb", bufs=1) as r_p,
            tc.tile_pool(name="ctxsb", bufs=2) as ctx_p,
            tc.tile_pool(name="osb", bufs=3) as o_p,
            tc.tile_pool(name="psbig", bufs=2, space="PSUM") as psbig_p,
            tc.tile_pool(name="pso", bufs=5, space="PSUM") as pso_p,
            tc.tile_pool(name="psmall", bufs=1, space="PSUM") as psmall_p,
        ):
            # pass-1 consts first: their DMAs go ahead of the fat pass-2 weights
            kb_sb = consts.tile([1, 2, Cm], f8)
            nc.sync.dma_start(kb_sb, kb_e[:])
            kvw_sb = consts.tile([P, KO, 512], f8)
            for o2 in range(KO // 2):   # pair-granular so conv starts early
                nc.sync.dma_start(kvw_sb[:, 2 * o2:2 * o2 + 2, :],
                                  kvw_e[:][:, 2 * o2:2 * o2 + 2, :])
            ind_sb = consts.tile([P, NPIX // P, M2], f8)
            nc.gpsimd.dma_start(ind_sb[:, 0:16, :], ind_e[:][:, 0:16, :])
            nc.gpsimd.dma_start(ind_sb[:, 16:32, :], ind_e[:][:, 16:32, :])
            ar_sb = consts.tile([M, 1], f32)
            nc.gpsimd.dma_start(ar_sb, ar_e[:])
            qw_sb = consts.tile([P, QO, Cm], f8)
            nc.gpsimd.dma_start(qw_sb, qw_e[:])
            qb_sb = consts.tile([P, 2], f32)
            nc.gpsimd.dma_start(qb_sb, qb_e[:])

            ones8 = consts.tile([1, 2, P], f8)    # K-bias DR seed lhsT
            nc.vector.memset(ones8[:, 0, :], 1.0)
            nc.vector.memset(ones8[:, 1, :], 0.0)
            ones1m = consts.tile([1, M], bf)      # psr broadcast lhsT
            nc.vector.memset(ones1m, 1.0)
            ones_m = consts.tile([M, 1], bf)      # denominator lhsT
            nc.vector.memset(ones_m, 1.0)
            ident = consts.tile([P, P], f32)
            make_identity(nc, ident)

            kvpool_f32 = consts.tile([M, 512], f32)
            v_poolT = consts.tile([M, Cm], bf)
            k_pool8 = consts.tile([P, 2, M2], f8)
            nc.vector.memset(k_pool8, 0.0)

            # ---------------- pass 1: low -> pooled K/V (fp8 DR) ----------------
            pool_acc = pso_p.tile([M2, 512], f32, name="pool_acc", tag="o")
            kvt8 = None
            kvt_gate = None
            for dt_ in range(NT):
                lo8 = lobf_p.tile([P, KO, PIX_T], f8)
                if dt_ == 0:   # tile 0 pair-split on the scalar queue
                    for o2 in range(KO // 2):
                        nc.scalar.dma_start(lo8[:, 2 * o2:2 * o2 + 2, :],
                                            low_e[:][:, 0, 2 * o2:2 * o2 + 2, :])
                elif dt_ % 2 == 1:   # alternate queues: 2x streaming bandwidth
                    nc.scalar.dma_start(lo8, low_e[:][:, dt_])
                else:
                    nc.sync.dma_start(lo8, low_e[:][:, dt_])
                for tt in range(PC):
                    t = dt_ * PC + tt
                    ps = psbig_p.tile([P, 512], f32, tag="big")
                    for o2 in range(KO // 2):
                        nc.tensor.matmul(
                            ps, lo8[:, 2 * o2:2 * o2 + 2, tt * P:(tt + 1) * P],
                            kvw_sb[:, 2 * o2:2 * o2 + 2, :],
                            start=(o2 == 0), stop=(o2 == KO // 2 - 1),
                            perf_mode=DR, skip_group_check=True)
                        if o2 == 0:   # K-bias into the zeroed psum (cols 0:256)
                            nc.tensor.matmul(ps[:, 0:Cm], ones8[:, 0:2, :],
                                             kb_sb[:, 0:2, :],
                                             start=False, stop=False, perf_mode=DR,
                                             skip_group_check=True)
                    half = t % 2
                    if half == 0:
                        kvt8 = kvt_p.tile([P, 2, 512], f8)
                    nc.scalar.activation(kvt8[:, half, 0:Cm], ps[:, 0:Cm], ACT.Relu)
                    nc.vector.tensor_copy(kvt8[:, half, Cm:512], ps[:, Cm:512])
                    if half == 1:
                        nc.tensor.matmul(pool_acc, ind_sb[:, t - 1:t + 1, :], kvt8,
                                         start=(t == 1), stop=(t == NPIX // P - 1),
                                         perf_mode=DR, skip_group_check=True)
                        if t == 11:
                            kvt_gate = kvt8

            # prefetch tile-0 high (bf16 on sync q, fp8 on vector q)
            hi_tiles, hi8_tiles = {}, {}

            def stage_hi(t):
                hi_bf = hibf_p.tile([P, QO, PIX_T], bf)
                if t == 0:   # halves so tile-0 W2 opens start sooner
                    nc.sync.dma_start(hi_bf[:, 0:QO // 2, :],
                                      high_e[:][:, 0, 0:QO // 2, :])
                    nc.sync.dma_start(hi_bf[:, QO // 2:QO, :],
                                      high_e[:][:, 0, QO // 2:QO, :])
                else:
                    nc.sync.dma_start(hi_bf, high_e[:][:, t])
                hi_tiles[t] = hi_bf

            def cast_hi8(t):
                # on-chip bf16 -> fp8 cast for the q conv: saves 8 MB of DMA
                hi_bf = hi_tiles[t]
                hi8 = hi8_p.tile([P, QO, PIX_T], f8)
                nc.vector.tensor_copy(hi8[:, 0:QO // 2, :], hi_bf[:, 0:QO // 2, :])
                nc.vector.tensor_copy(hi8[:, QO // 2:QO, :], hi_bf[:, QO // 2:QO, :])
                hi8_tiles[t] = hi8

            stage_hi(0)
            cast_hi8(0)

            # fat pass-2 weights: qw8/at first (needed at tile-0 start), then
            # W2 streamed per oc chunk so tile-0 opens consume them as they land
            at_sb = consts.tile([P, 2, Co], f8)
            nc.gpsimd.dma_start(at_sb, at_e[:])
            w2gate = consts.tile([1, 1], f8)
            nc.gpsimd.tensor_copy(w2gate, kvt_gate[0:1, 0, 0:1])
            w2_sb = consts.tile([P, OC, QO, P], bf)
            for oc in range(2):
                nc.gpsimd.dma_start(w2_sb[:, oc], w2_e[:][:, oc])
            bo_sb = consts.tile([P, OC], f32)
            nc.gpsimd.dma_start(bo_sb, bo_e[:])

            def emit_q(t):
                """fp8 DR q conv for tile t -> q/4 in fp8 [P, 2, PIX_T]."""
                hi8 = hi8_tiles[t]
                q8 = q_p.tile([P, 2, PIX_T], f8)
                for j in range(2):
                    psq = psbig_p.tile([P, PIX_T], f32, tag="big")
                    for o2 in range(QO // 2):
                        nc.tensor.matmul(
                            psq, qw_sb[:, 2 * o2:2 * o2 + 2, j * P:(j + 1) * P],
                            hi8[:, 2 * o2:2 * o2 + 2, :],
                            start=(o2 == 0), stop=(o2 == QO // 2 - 1),
                            perf_mode=DR)
                    nc.scalar.activation(q8[:, j, :], psq, ACT.Relu,
                                         bias=qb_sb[:, j:j + 1], scale=1.0 / (4.0 * QWS))
                return q8

            q_next = emit_q(0)
            # rest of W2 gated on tile-0 q conv: keeps the 15-30us DMA window
            # free for hi8[0]/hi_bf[0], while W2 still lands ahead of its opens
            w2gate2 = consts.tile([1, 1], f8)
            nc.gpsimd.tensor_copy(w2gate2, q_next[0:1, 0, 0:1])
            for oc in range(2, OC):
                nc.gpsimd.dma_start(w2_sb[:, oc], w2_e[:][:, oc])

            # epilogue: scale by 1/area, split V (bf16) / K (fp8 at k/4 via
            # transpose + scaled copy); PE transposes overlap tile-0 q conv
            nc.vector.tensor_scalar_mul(kvpool_f32, pool_acc[0:M, :], ar_sb)
            nc.scalar.activation(v_poolT, kvpool_f32[:, Cm:512], ACT.Copy)
            for j in range(2):
                pst = psbig_p.tile([P, M], f32, tag="big")
                nc.tensor.transpose(pst, kvpool_f32[:, j * P:(j + 1) * P],
                                    ident[:M, :M])
                nc.scalar.activation(k_pool8[:, j, 0:M], pst, ACT.Identity,
                                     scale=1.0 / (4.0 * KSC))

            # ---------------- pass 2: per pixel tile ----------------
            for t in range(NT):
                sl = slice(t * PIX_T, (t + 1) * PIX_T)
                hi_bf = hi_tiles.pop(t)
                hi8_tiles.pop(t)
                if t + 1 < NT:
                    stage_hi(t + 1)
                stage_t = t
                q8 = q_next

                psim = psmall_p.tile([M2, PIX_T], f32, tag="s")
                nc.tensor.matmul(psim, k_pool8[:, 0:2, :], q8[:, 0:2, :],
                                 start=True, stop=True, perf_mode=DR,
                                 skip_group_check=True)
                e_sb = e_p.tile([M, PIX_T], bf)
                nc.scalar.activation(e_sb, psim[0:M, :], ACT.Exp)

                pso_tiles = {}

                def open_half(oc, hf, hi_bf=hi_bf):
                    if hf == 0:
                        pso_tiles[oc] = pso_p.tile([P, PIX_T], f32, name="pso", tag="o")
                    pso = pso_tiles[oc]
                    for o in range(hf * (QO // 2), (hf + 1) * (QO // 2)):
                        nc.tensor.matmul(pso, w2_sb[:, oc, o, :], hi_bf[:, o, :],
                                         start=(o == 0), stop=False,
                                         skip_group_check=True)

                def close_group(oc, sl=sl):
                    pso = pso_tiles.pop(oc)
                    ctx_sb = ctx_cur[0]
                    nc.tensor.matmul(pso, at_sb[:, 0:2, oc * P:(oc + 1) * P],
                                     ctx_sb[:, 0:2, :],
                                     start=False, stop=True, perf_mode=DR,
                                     skip_group_check=True)
                    o_sb = o_p.tile([P, PIX_T], f16)
                    if oc % 2 == 1:
                        nc.scalar.activation(o_sb, pso, ACT.Identity,
                                             bias=bo_sb[:, oc:oc + 1], scale=1.0 / WSC)
                    else:
                        nc.vector.tensor_scalar(o_sb, pso, 1.0 / WSC,
                                                bo_sb[:, oc:oc + 1],
                                                mybir.AluOpType.mult,
                                                mybir.AluOpType.add)
                    nc.sync.dma_start(out_r[oc][:, sl], o_sb)

                open_half(0, 0)
                psd = psmall_p.tile([1, PIX_T], f32, tag="s")
                nc.tensor.matmul(psd, ones_m, e_sb, start=True, stop=True,
                                 skip_group_check=True)
                open_half(0, 1)
                open_half(1, 0)
                r_sb = r_p.tile([1, PIX_T], f32)
                nc.vector.reciprocal_approx_fast(out=r_sb, in_=psd)
                r_bf = r_p.tile([1, PIX_T], bf, name="r_bf")
                nc.vector.tensor_copy(r_bf, r_sb)
                psr = psmall_p.tile([M, PIX_T], f32, tag="s")
                nc.tensor.matmul(psr, ones1m, r_bf, start=True, stop=True,
                                 skip_group_check=True)
                open_half(1, 1)
                open_half(2, 0)
                en_sb = en_p.tile([M, PIX_T], bf)
                nc.vector.tensor_mul(en_sb, e_sb, psr)
                ctx_sb = ctx_p.tile([P, 2, PIX_T], f8)
                ctx_cur = [ctx_sb]
                psc0 = psbig_p.tile([P, PIX_T], f32, tag="big")
                nc.tensor.matmul(psc0, v_poolT[:, 0:P], en_sb,
                                 start=True, stop=True, skip_group_check=True)
                psc1 = psbig_p.tile([P, PIX_T], f32, tag="big")
                nc.tensor.matmul(psc1, v_poolT[:, P:2 * P], en_sb,
                                 start=True, stop=True, skip_group_check=True)
                open_half(2, 1)
                open_half(3, 0)
                nc.scalar.activation(ctx_sb[:, 0, :], psc0, ACT.Copy)
                nc.vector.tensor_copy(ctx_sb[:, 1, :], psc1)
                if stage_t + 1 < NT:
                    cast_hi8(stage_t + 1)
                open_half(3, 1)
                open_half(4, 0)
                open_half(4, 1)
                close_group(0)
                close_group(1)
                # q conv for the next tile: independent PE work that also
                # covers close(0)/(1)'s psum drain before bank reuse
                if t + 1 < NT:
                    q_next = emit_q(t + 1)
                open_half(5, 0)
                open_half(5, 1)
                for i in range(2, OC - 4):
                    close_group(i)
                    if i + 4 < OC:
                        open_half(i + 4, 0)
                        open_half(i + 4, 1)
                for oc in range(OC - 4, OC):
                    close_group(oc)
    nc.finalize()
    return nc


def kernel(**inputs):
    global _cached, _last_results
    if _cached is None:
        _cached = build_bass()
    nc = _cached
    wts = _prep_weights(inputs)
    # pack [C, H*W] -> [p, tile, o, pix] so each per-tile DMA is contiguous
    low = np.ascontiguousarray(
        np.asarray(inputs["low_feats"], F32).reshape(N_CORES, KO, P, NT, PIX_T)
        .transpose(0, 2, 3, 1, 4).astype(F8))
    high_f = (np.asarray(inputs["high_feats"], F32)
              .reshape(N_CORES, QO, P, NT, PIX_T).transpose(0, 2, 3, 1, 4))
    high = np.ascontiguousarray(high_f.astype(BF))
    in_maps = [dict(wts, low=low[i], high=high[i])
               for i in range(N_CORES)]
    res = run_bass_kernel_spmd(nc, in_maps, core_ids=list(range(N_CORES)))
    _last_results = res
    out = np.stack([res.results[i]["out"] for i in range(N_CORES)])
    return out.reshape(N_CORES, Co, H, W).astype(F32)


if __name__ == "__main__":
    rng = np.random.default_rng(0)
    dummy = {
        "low_feats": rng.standard_normal((8, Cl, H, W), dtype=np.float32),
        "high_feats": rng.standard_normal((8, Ch, H, W), dtype=np.float32),
    }
    for k, shape in [("q_w", (Cm, Ch)), ("k_w", (Cm, Cl)), ("v_w", (Cm, Cl)),
                     ("o_w", (Co, Cm)), ("bn_w", (Co, Co + Ch))]:
        dummy[k] = rng.standard_normal(shape, dtype=np.float32) * 0.02
    for k in ["q_g", "q_v", "k_g", "k_v"]:
        dummy[k] = rng.uniform(0.5, 1.5, Cm).astype(np.float32)
    for k in ["q_b", "q_m", "k_b", "k_m", "v_b"]:
        dummy[k] = rng.standard_normal(Cm).astype(np.float32) * 0.1
    for k in ["bn_g", "bn_v"]:
        dummy[k] = rng.uniform(0.5, 1.5, Co).astype(np.float32)
    for k in ["bn_b", "bn_m", "o_b"]:
        dummy[k] = rng.standard_normal(Co).astype(np.float32) * 0.1
    out = kernel(**dummy)
    print("out", out.shape, out.dtype)
